# revision 35
# baseline (speedup 1.0000x reference)
"""MultiHeadAttention (RoPE, causal) Trainium2 kernel over 8 NeuronCores.

Sharding: batch (2) x head-groups (4 heads each) -> 8 cores.

Single merged pipeline over s-blocks (512 wide).  Per block sb:
  - load x tiles once (shared by Q, K and V sweeps)
  - Q^T,K^T projections + RoPE (ACT half-swap from psum, DVE mul/add)
  - V projection -> vt s-tiles (causal: only k-tiles <= sb are needed
    by the attention blocks that run this iteration)
  - attention blocks (h=0..3, j=sb): S^T = K^T~Q^T, exp on ACT, causal
    triangle mask, O^T accumulation; denominators via DVE-summed
    U = sum_i P_i folded by an M=1 matmul; reciprocal_approx_fast;
    partition-broadcast via K=1 matmul, normalization multiply.  The
    broadcast of each block is deferred past the next block's matmuls
    so the in-order PE queue never waits on a reciprocal.
  - output projection for this block's 4 q-tiles + fp16 z store.
Interleaving the phases hides the ACT-bound exp stream and the
DVE-bound denominator sums under the PE-bound projection sweeps.

Numerics: fp16 operands everywhere (PE full rate, half DMA), psum
accumulation fp32; exp<=~250 and row-sums ~3.5k fit fp16 comfortably.
Host sums the 4 per-core partials per batch.
"""

import sys

if "/opt/trn_rl_repo" not in sys.path:
    sys.path.insert(0, "/opt/trn_rl_repo")

import numpy as np

EMBED = 2048
S = 2048
NH = 16
HD = 128
B = 2
N_CORES = 8
HPC = 4              # heads per core
CW = HPC * HD        # 512: per-core projection width
SBK = 512            # s block width
NSB = S // SBK       # 4
NEC = EMBED // 128   # 16 e-chunks
NST = S // 128       # 16 s tiles / q tiles / k tiles
ROPE_BASE = 10000.0
SCALE = 1.0 / float(np.sqrt(HD))

_CACHE = {}


def _build_program():
    import concourse.bacc as bacc
    import concourse.mybir as mybir
    import concourse.tile as tile

    f32 = mybir.dt.float32
    f32r = mybir.dt.float32r
    f16 = mybir.dt.float16
    EXP = mybir.ActivationFunctionType.Exp

    nc = bacc.Bacc("TRN2", target_bir_lowering=False, debug=False,
                   num_devices=N_CORES)

    xt_d = nc.dram_tensor("xt", [EMBED, S], f16, kind="ExternalInput").ap()
    wq_d = nc.dram_tensor("wq", [EMBED, CW], f16, kind="ExternalInput").ap()
    wk_d = nc.dram_tensor("wk", [EMBED, CW], f16, kind="ExternalInput").ap()
    wv_d = nc.dram_tensor("wv", [EMBED, CW], f16, kind="ExternalInput").ap()
    wo_d = nc.dram_tensor("wo", [CW, EMBED], f16, kind="ExternalInput").ap()
    cos_d = nc.dram_tensor("cost", [HD, S], f16, kind="ExternalInput").ap()
    sin_d = nc.dram_tensor("sints", [HD, S], f16, kind="ExternalInput").ap()
    msk_d = nc.dram_tensor("masks", [128, 256], f16, kind="ExternalInput").ap()
    on32_d = nc.dram_tensor("ones32", [1, 128], f32, kind="ExternalInput").ap()
    z_d = nc.dram_tensor("z", [S, EMBED], f16, kind="ExternalOutput").ap()

    XG = 8           # x tiles per s-block (2 e-chunks each)
    XW = 2 * SBK     # x tile width

    with tile.TileContext(nc, pool_alloc_mode="queue") as tc, \
         nc.allow_low_precision(reason="fp16 attention pipeline"):
        pp = tc.alloc_tile_pool(name="persist", bufs=1)
        ps = tc.alloc_tile_pool(name="ps", bufs=8, space="PSUM")
        qt = pp.tile([128, HPC * S], f16, tag="qt")   # Q^T rope, per head
        kt = pp.tile([128, HPC * S], f16, tag="kt")   # K^T rope, per head
        ot = pp.tile([128, HPC * S], f16, tag="ot")   # unnorm/norm O^T
        xa = tc.alloc_tile_pool(name="xa", bufs=XG)
        wvp = tc.alloc_tile_pool(name="wv", bufs=1)
        wv_sb = wvp.tile([128, NEC * CW], f16, tag="wv")
        wp = tc.alloc_tile_pool(name="wqk", bufs=1)
        wq_sb = wp.tile([128, NEC * CW], f16, tag="wq")
        wk_sb = wp.tile([128, NEC * CW], f16, tag="wk")
        wop = tc.alloc_tile_pool(name="wo", bufs=1)
        wo_sb = wop.tile([128, HPC * EMBED], f16, tag="wo")
        cs = tc.alloc_tile_pool(name="cossin", bufs=2)
        rp = tc.alloc_tile_pool(name="ropetmp", bufs=2)
        lrp = tc.alloc_tile_pool(name="lrec", bufs=2)
        ptp = tc.alloc_tile_pool(name="pts", bufs=8)
        upl = tc.alloc_tile_pool(name="usum", bufs=2)
        zp = tc.alloc_tile_pool(name="zsb", bufs=3)
        vmp = tc.alloc_tile_pool(name="vtmsk", bufs=1, side="right")
        vt = vmp.tile([128, NST * CW], f16, tag="vt")
        msk_sb = vmp.tile([128, 256], f16, tag="msk")
        on32_sb = vmp.tile([1, 128], f32r, tag="on32")
        tri = msk_sb[:, 0:128]          # lower-triangle (k <= q') [128,128]
        ones_col = msk_sb[:, 128:129]   # all-ones [128,1]
        ones_row = msk_sb[0:1, 128:256]  # all-ones [1,128]

        def rope(psrc, dst, cos_sb, sin_sb, sb, h, nm, swap_eng):
            # half-swap from psum (exempt from the SBUF partition-pairing
            # rule); Q-ropes swap on ACT, K-ropes on DVE so the ACT queue is
            # clear for the attention exps that follow the K sweep
            sw = rp.tile([128, SBK], f16, tag="sw", name=f"sw{nm}{sb}_{h}")
            t1 = rp.tile([128, SBK], f16, tag="t1", name=f"t1{nm}{sb}_{h}")
            t2 = rp.tile([128, SBK], f16, tag="t2", name=f"t2{nm}{sb}_{h}")
            if swap_eng == "act":
                nc.scalar.copy(sw[0:64, :], psrc[64:128, :])
                nc.scalar.copy(sw[64:128, :], psrc[0:64, :])
            else:
                nc.vector.tensor_copy(sw[0:64, :], psrc[64:128, :])
                nc.vector.tensor_copy(sw[64:128, :], psrc[0:64, :])
            nc.vector.tensor_mul(t1[:], psrc[:], cos_sb[:])
            nc.vector.tensor_mul(t2[:], sw[:], sin_sb[:])
            ds = slice(h * S + sb * SBK, h * S + (sb + 1) * SBK)
            nc.vector.tensor_add(dst[:, ds], t1[:], t2[:])

        def emit_w_chunk(dst_sb, src_d, g):
            nc.gpsimd.dma_start(
                dst_sb[:, g * 2 * CW:(g + 1) * 2 * CW].rearrange(
                    "p (c m) -> p c m", m=CW),
                src_d[g * 256:(g + 1) * 256, :].rearrange(
                    "(c p) m -> p c m", p=128))

        pending = []  # deferred (lr, qs) epilogues, flushed one block later

        def flush_epilogue():
            for lr_p, qs_p, nm in pending:
                bcps = ps.tile([128, SBK], f32, tag="ps", name=f"bc{nm}")
                nc.tensor.matmul(bcps[:], lhsT=ones_row, rhs=lr_p[:],
                                 start=True, stop=True)
                nc.vector.tensor_mul(ot[:, qs_p], ot[:, qs_p], bcps[:])
            pending.clear()

        d_queue = []   # (q_i, eb, h) output-projection steps, one matmul each
        dstate = {}

        def emit_d_step(step):
            q_i, eb, h = step
            if eb == 0 and h == 0:
                dstate["z_sb"] = zp.tile([128, EMBED], f16, tag="zs",
                                         name=f"zs{q_i}")
            if h == 0:
                dstate["zps"] = ps.tile([128, SBK], f32, tag="ps",
                                        name=f"z{q_i}_{eb}")
            nc.tensor.matmul(
                dstate["zps"][:],
                lhsT=ot[:, h * S + q_i * 128:h * S + (q_i + 1) * 128],
                rhs=wo_sb[:, h * EMBED + eb * SBK:h * EMBED + (eb + 1) * SBK],
                start=(h == 0), stop=(h == HPC - 1))
            if h == HPC - 1:
                ebs = slice(eb * SBK, (eb + 1) * SBK)
                if eb % 2 == 0:
                    nc.vector.tensor_copy(dstate["z_sb"][:, ebs],
                                          dstate["zps"][:])
                else:
                    nc.scalar.copy(dstate["z_sb"][:, ebs], dstate["zps"][:])
                if q_i == NST - 1:
                    # last tile: store per chunk so the final drain is short
                    nc.sync.dma_start(z_d[q_i * 128:(q_i + 1) * 128, ebs],
                                      dstate["z_sb"][:, ebs])
                elif eb == 3:
                    nc.sync.dma_start(z_d[q_i * 128:(q_i + 1) * 128, :],
                                      dstate["z_sb"][:])

        for sb in range(NSB):
            ss = slice(sb * SBK, (sb + 1) * SBK)
            # ---- x tiles, loaded once and shared by Q/K/V sweeps ----
            xts = []
            for g in range(XG):
                xt_g = xa.tile([128, XW], f16, tag="x", name=f"x{sb}_{g}")
                if sb == 0 and g == 0:
                    # split the first loads so the first matmul starts early
                    for c in range(2):
                        nc.sync.dma_start(
                            xt_g[:, c * SBK:(c + 1) * SBK],
                            xt_d[c * 128:(c + 1) * 128, ss])
                        nc.gpsimd.dma_start(
                            wq_sb[:, c * CW:(c + 1) * CW],
                            wq_d[c * 128:(c + 1) * 128, :])
                else:
                    src_ap = xt_d[g * 256:(g + 1) * 256, ss]
                    nc.sync.dma_start(
                        xt_g[:].rearrange("p (c s) -> p c s", s=SBK),
                        src_ap.rearrange("(c p) s -> p c s", p=128))
                    if sb == 0:
                        emit_w_chunk(wq_sb, wq_d, g)
                xts.append(xt_g)
            cos_sb = cs.tile([128, SBK], f16, tag="cos", name=f"cos{sb}")
            sin_sb = cs.tile([128, SBK], f16, tag="sin", name=f"sin{sb}")
            nc.gpsimd.dma_start(cos_sb[:], cos_d[:, ss])
            nc.gpsimd.dma_start(sin_sb[:], sin_d[:, ss])
            if sb == 0:
                nc.gpsimd.dma_start(msk_sb[:], msk_d[:])
                nc.gpsimd.dma_start(on32_sb[:], on32_d[:].bitcast(f32r))
            # -------- Q sweep --------
            qp = [ps.tile([128, SBK], f32, tag="ps", name=f"qp{sb}_{_h}")
                  for _h in range(HPC)]
            for g in range(XG):
                for el in range(2):
                    ec = 2 * g + el
                    st, sp = (ec == 0), (ec == NEC - 1)
                    xv = xts[g][:, el * SBK:(el + 1) * SBK]
                    for h in range(HPC):
                        wsl = slice(ec * CW + h * HD, ec * CW + (h + 1) * HD)
                        nc.tensor.matmul(qp[h][:], lhsT=wq_sb[:, wsl],
                                         rhs=xv, start=st, stop=sp)
            for h in range(HPC):
                rope(qp[h], qt, cos_sb, sin_sb, sb, h, "q", "act")
            # -------- K sweep --------
            kp = [ps.tile([128, SBK], f32, tag="ps", name=f"kp{sb}_{_h}")
                  for _h in range(HPC)]
            for g in range(XG):
                if sb == 0:
                    emit_w_chunk(wk_sb, wk_d, g)
                for el in range(2):
                    ec = 2 * g + el
                    st, sp = (ec == 0), (ec == NEC - 1)
                    xv = xts[g][:, el * SBK:(el + 1) * SBK]
                    for h in range(HPC):
                        wsl = slice(ec * CW + h * HD, ec * CW + (h + 1) * HD)
                        nc.tensor.matmul(kp[h][:], lhsT=wk_sb[:, wsl],
                                         rhs=xv, start=st, stop=sp)
            for h in range(HPC):
                rope(kp[h], kt, cos_sb, sin_sb, sb, h, "k", "dve")
            # -------- V sweep (same x tiles) --------
            vp = [ps.tile([128, CW], f32, tag="ps", name=f"vp{sb}_{_s}")
                  for _s in range(4)]
            for g in range(XG):
                if sb == 0:
                    emit_w_chunk(wv_sb, wv_d, g)
                for el in range(2):
                    ec = 2 * g + el
                    st, sp = (ec == 0), (ec == NEC - 1)
                    for sub in range(4):
                        nc.tensor.matmul(
                            vp[sub][:],
                            lhsT=xts[g][:, el * SBK + sub * 128:
                                        el * SBK + (sub + 1) * 128],
                            rhs=wv_sb[:, ec * CW:(ec + 1) * CW],
                            start=st, stop=sp)
            for sub in range(4):
                stile = sb * 4 + sub
                dsl = vt[:, stile * CW:(stile + 1) * CW]
                if sub % 2 == 0:
                    nc.scalar.copy(dsl, vp[sub][:])
                else:
                    nc.vector.tensor_copy(dsl, vp[sub][:])
            if sb == 0:
                for h in range(HPC):
                    nc.gpsimd.dma_start(
                        wo_sb[:, h * EMBED:(h + 1) * EMBED],
                        wo_d[h * 128:(h + 1) * 128, :])

            # -------- attention blocks (h = 0..3, j = sb) --------
            # the previous iteration's output projection (d_queue) is
            # interleaved between attention tiles: per tile the PE's S+AV
            # (~430ns) trails the exp on ACT (~570ns), so without filler
            # the PE drains its psum-ring lead and stalls at ACT pace
            j = sb
            nkt = 4 * j + 4  # causal: k tiles 0..4j+3
            # the pending h=3 epilogue of iteration sb-1 must flush before
            # any interleaved D matmul touches that head's normalized O^T
            # (its broadcast would otherwise sit behind the reader in the
            # in-order PE queue)
            flush_epilogue()
            d_ratio = len(d_queue) / float(HPC * nkt) if d_queue else 0.0
            d_credit = 0.0
            for h in range(HPC):
                avp = ps.tile([128, SBK], f32, tag="ps", name=f"av{h}_{j}")
                u_sb = upl.tile([128, SBK], f16, tag="u", name=f"u{h}_{j}")
                qs = slice(h * S + j * SBK, h * S + (j + 1) * SBK)
                pt_last = None
                pend_av = []  # software-pipeline AV behind S

                def do_av(pi, pq0, ppt):
                    nonlocal pt_last
                    nc.tensor.matmul(
                        avp[:, pq0:SBK],
                        lhsT=vt[:, pi * CW + h * HD:pi * CW + (h + 1) * HD],
                        rhs=ppt[:, pq0:SBK],
                        start=(pi == 0), stop=(pi == nkt - 1))
                    # fold P into the running denominator sum on DVE; the
                    # last (128-wide) diagonal tile goes straight into a
                    # second l-matmul instead, off the serial DVE chain
                    if pi == 0:
                        nc.vector.tensor_copy(u_sb[:], ppt[:])
                    elif pi < nkt - 1:
                        nc.vector.tensor_add(u_sb[:, pq0:SBK],
                                             u_sb[:, pq0:SBK],
                                             ppt[:, pq0:SBK])
                    else:
                        pt_last = ppt

                for i in range(nkt):
                    o_idx = i - 4 * j
                    # crossing tiles: only q >= 128*o_idx is unmasked
                    q0 = 128 * o_idx if o_idx > 0 else 0
                    sp_t = ps.tile([128, SBK], f32, tag="ps",
                                   name=f"s{h}_{j}_{i}")
                    ks = slice(h * S + i * 128, h * S + (i + 1) * 128)
                    nc.tensor.matmul(sp_t[:, q0:SBK], lhsT=kt[:, ks],
                                     rhs=qt[:, qs.start + q0:qs.stop],
                                     start=True, stop=True)
                    pt_sb = ptp.tile([128, SBK], f16, tag="p",
                                     name=f"p{h}_{j}_{i}")
                    nc.scalar.activation(pt_sb[:, q0:SBK], sp_t[:, q0:SBK],
                                         EXP, scale=SCALE)
                    if o_idx >= 0:  # mask the diagonal 128-band
                        nc.vector.tensor_mul(
                            pt_sb[:, q0:q0 + 128], pt_sb[:, q0:q0 + 128], tri)
                    if len(pend_av) == 2:  # AV runs two tiles behind S
                        do_av(*pend_av.pop(0))
                    pend_av.append((i, q0, pt_sb))
                    d_credit += d_ratio
                    while d_queue and d_credit >= 1.0:
                        emit_d_step(d_queue.pop(0))
                        d_credit -= 1.0
                for pa in pend_av:
                    do_av(*pa)
                # denominators: fold partitions into a psum row
                lp = ps.tile([1, SBK], f32, tag="ps", name=f"l{h}_{j}")
                nc.tensor.matmul(lp[:], lhsT=ones_col, rhs=u_sb[:],
                                 start=True, stop=False)
                nc.tensor.matmul(lp[:, 384:SBK], lhsT=ones_col,
                                 rhs=pt_last[:, 384:SBK],
                                 start=False, stop=True)
                lr = lrp.tile([1, SBK], f32, tag="lr", name=f"lr{h}_{j}")
                nc.vector.reciprocal_approx_fast(lr[:], lp[:])
                lrh = lrp.tile([1, SBK], f16, tag="lrh", name=f"lrh{h}_{j}")
                nc.vector.tensor_copy(lrh[:], lr[:])
                nc.vector.tensor_copy(ot[:, qs], avp[:])
                # broadcast+normalize of the PREVIOUS block runs here so its
                # PE matmul never waits on this block's reciprocal
                flush_epilogue()
                pending.append((lrh, qs, f"{h}_{j}"))

            # -------- output projection --------
            # drain anything left of D(sb-1), then queue D(sb) for
            # interleaving into the next iteration's attention tiles; the
            # last iteration's D runs here directly (no C to hide it in)
            while d_queue:
                emit_d_step(d_queue.pop(0))
            if sb < NSB - 1:
                d_queue = [(q_i, eb, h)
                           for q_i in range(4 * sb, 4 * sb + 4)
                           for eb in range(4)
                           for h in range(HPC)]
            else:
                for q_i in range(4 * sb, 4 * sb + 4):
                    for eb in range(4):
                        for h in range(HPC):
                            if h == HPC - 1 and pending:
                                flush_epilogue()
                            emit_d_step((q_i, eb, h))

        vmp.release()
        zp.release()
        upl.release()
        ptp.release()
        lrp.release()
        rp.release()
        cs.release()
        wop.release()
        wp.release()
        wvp.release()
        xa.release()
        pp.release()
        ps.release()

    nc.compile()
    return nc


def _host_tables():
    inv_freq = 1.0 / (ROPE_BASE ** (np.arange(0, HD, 2, dtype=np.float64) / HD))
    ang = np.arange(S, dtype=np.float64)[:, None] * inv_freq[None, :]  # [S, 64]
    cos = np.cos(ang)
    sin = np.sin(ang)
    cost = np.ascontiguousarray(
        np.concatenate([cos, cos], axis=1).T.astype(np.float16))  # [128, S]
    sints = np.ascontiguousarray(
        np.concatenate([-sin, sin], axis=1).T.astype(np.float16))
    kk = np.arange(128)[:, None]
    rr = np.arange(128)[None, :]
    masks = np.ones((128, 256), dtype=np.float16)
    masks[:, 0:128] = (kk <= rr).astype(np.float16)
    return cost, sints, masks


def _in_maps(x, Wq, Wk, Wv, Wo):
    cost, sints, masks = _host_tables()
    ones32 = np.ones((1, 128), dtype=np.float32)
    maps = []
    for c in range(N_CORES):
        b = c // 4
        h0 = (c % 4) * CW  # column offset of this core's 4 heads
        maps.append({
            "xt": np.ascontiguousarray(x[b].T.astype(np.float16)),
            "wq": np.ascontiguousarray(Wq[:, h0:h0 + CW].astype(np.float16)),
            "wk": np.ascontiguousarray(Wk[:, h0:h0 + CW].astype(np.float16)),
            "wv": np.ascontiguousarray(Wv[:, h0:h0 + CW].astype(np.float16)),
            "wo": np.ascontiguousarray(Wo[h0:h0 + CW, :].astype(np.float16)),
            "cost": cost,
            "sints": sints,
            "masks": masks,
            "ones32": ones32,
        })
    return maps


def kernel(x, Wq, Wk, Wv, Wo):
    from concourse.bass_utils import run_bass_kernel_spmd

    x = np.asarray(x, dtype=np.float32)
    Wq = np.asarray(Wq, dtype=np.float32)
    Wk = np.asarray(Wk, dtype=np.float32)
    Wv = np.asarray(Wv, dtype=np.float32)
    Wo = np.asarray(Wo, dtype=np.float32)

    if "nc" not in _CACHE:
        _CACHE["nc"] = _build_program()
    nc = _CACHE["nc"]

    res = run_bass_kernel_spmd(nc, _in_maps(x, Wq, Wk, Wv, Wo),
                               core_ids=list(range(N_CORES)))
    zs = [np.asarray(res.results[c]["z"], dtype=np.float32)
          for c in range(N_CORES)]
    out = np.empty((B, S, EMBED), dtype=np.float32)
    out[0] = zs[0] + zs[1] + zs[2] + zs[3]
    out[1] = zs[4] + zs[5] + zs[6] + zs[7]
    return out


# revision 44
# speedup vs baseline: 1.0213x; 1.0213x over previous
"""MultiHeadAttention (RoPE, causal) Trainium2 kernel over 8 NeuronCores.

Sharding: batch (2) x head-groups (4 heads each) -> 8 cores.

Single merged pipeline over s-blocks (512 wide).  Per block sb:
  - load x tiles once (shared by Q, K and V sweeps)
  - Q^T,K^T projections + RoPE (ACT half-swap from psum, DVE mul/add)
  - V projection -> vt s-tiles (causal: only k-tiles <= sb are needed
    by the attention blocks that run this iteration)
  - attention blocks (h=0..3, j=sb): S^T = K^T~Q^T, exp on ACT, causal
    triangle mask, O^T accumulation; denominators via DVE-summed
    U = sum_i P_i folded by an M=1 matmul; reciprocal_approx_fast;
    partition-broadcast via K=1 matmul, normalization multiply.  The
    broadcast of each block is deferred past the next block's matmuls
    so the in-order PE queue never waits on a reciprocal.
  - output projection for this block's 4 q-tiles + fp16 z store.
Interleaving the phases hides the ACT-bound exp stream and the
DVE-bound denominator sums under the PE-bound projection sweeps.

Numerics: fp16 operands everywhere (PE full rate, half DMA), psum
accumulation fp32; exp<=~250 and row-sums ~3.5k fit fp16 comfortably.
Host sums the 4 per-core partials per batch.
"""

import sys

if "/opt/trn_rl_repo" not in sys.path:
    sys.path.insert(0, "/opt/trn_rl_repo")

import numpy as np

EMBED = 2048
S = 2048
NH = 16
HD = 128
B = 2
N_CORES = 8
HPC = 4              # heads per core
CW = HPC * HD        # 512: per-core projection width
SBK = 512            # s block width
NSB = S // SBK       # 4
NEC = EMBED // 128   # 16 e-chunks
NST = S // 128       # 16 s tiles / q tiles / k tiles
ROPE_BASE = 10000.0
SCALE = 1.0 / float(np.sqrt(HD))

_CACHE = {}


def _build_program():
    import concourse.bacc as bacc
    import concourse.mybir as mybir
    import concourse.tile as tile

    f32 = mybir.dt.float32
    f32r = mybir.dt.float32r
    f16 = mybir.dt.float16
    EXP = mybir.ActivationFunctionType.Exp

    nc = bacc.Bacc("TRN2", target_bir_lowering=False, debug=False,
                   num_devices=N_CORES)

    xt_d = nc.dram_tensor("xt", [EMBED, S], f16, kind="ExternalInput").ap()
    wq_d = nc.dram_tensor("wq", [EMBED, CW], f16, kind="ExternalInput").ap()
    wk_d = nc.dram_tensor("wk", [EMBED, CW], f16, kind="ExternalInput").ap()
    wv_d = nc.dram_tensor("wv", [EMBED, CW], f16, kind="ExternalInput").ap()
    wo_d = nc.dram_tensor("wo", [CW, EMBED], f16, kind="ExternalInput").ap()
    cos_d = nc.dram_tensor("cost", [HD, S], f16, kind="ExternalInput").ap()
    sin_d = nc.dram_tensor("sints", [HD, S], f16, kind="ExternalInput").ap()
    msk_d = nc.dram_tensor("masks", [128, 256], f16, kind="ExternalInput").ap()
    on32_d = nc.dram_tensor("ones32", [1, 128], f32, kind="ExternalInput").ap()
    z_d = nc.dram_tensor("z", [S, EMBED], f16, kind="ExternalOutput").ap()

    XG = 8           # x tiles per s-block (2 e-chunks each)
    XW = 2 * SBK     # x tile width

    with tile.TileContext(nc, pool_alloc_mode="queue") as tc, \
         nc.allow_low_precision(reason="fp16 attention pipeline"):
        pp = tc.alloc_tile_pool(name="persist", bufs=1)
        ps = tc.alloc_tile_pool(name="ps", bufs=8, space="PSUM")
        qt = pp.tile([128, HPC * S], f16, tag="qt")   # Q^T rope, per head
        kt = pp.tile([128, HPC * S], f16, tag="kt")   # K^T rope, per head
        ot = pp.tile([128, HPC * S], f16, tag="ot")   # unnorm/norm O^T
        xa = tc.alloc_tile_pool(name="xa", bufs=2 * XG)
        wvp = tc.alloc_tile_pool(name="wv", bufs=1)
        wv_sb = wvp.tile([128, NEC * CW], f16, tag="wv")
        wp = tc.alloc_tile_pool(name="wqk", bufs=1)
        wq_sb = wp.tile([128, NEC * CW], f16, tag="wq")
        wk_sb = wp.tile([128, NEC * CW], f16, tag="wk")
        wop = tc.alloc_tile_pool(name="wo", bufs=1)
        wo_sb = wop.tile([128, HPC * EMBED], f16, tag="wo")
        cs = tc.alloc_tile_pool(name="cossin", bufs=2)
        rp = tc.alloc_tile_pool(name="ropetmp", bufs=2)
        lrp = tc.alloc_tile_pool(name="lrec", bufs=2)
        ptp = tc.alloc_tile_pool(name="pts", bufs=8)
        upl = tc.alloc_tile_pool(name="usum", bufs=2)
        zp = tc.alloc_tile_pool(name="zsb", bufs=2)
        vmp = tc.alloc_tile_pool(name="vtmsk", bufs=1, side="right")
        vt = vmp.tile([128, NST * CW], f16, tag="vt")
        msk_sb = vmp.tile([128, 256], f16, tag="msk")
        on32_sb = vmp.tile([1, 128], f32r, tag="on32")
        tri = msk_sb[:, 0:128]          # lower-triangle (k <= q') [128,128]
        ones_col = msk_sb[:, 128:129]   # all-ones [128,1]
        ones_row = msk_sb[0:1, 128:256]  # all-ones [1,128]

        def rope(psrc, dst, cos_sb, sin_sb, sb, h, nm, swap_eng):
            # half-swap from psum (exempt from the SBUF partition-pairing
            # rule); Q-ropes swap on ACT, K-ropes on DVE so the ACT queue is
            # clear for the attention exps that follow the K sweep
            sw = rp.tile([128, SBK], f16, tag="sw", name=f"sw{nm}{sb}_{h}")
            t1 = rp.tile([128, SBK], f16, tag="t1", name=f"t1{nm}{sb}_{h}")
            t2 = rp.tile([128, SBK], f16, tag="t2", name=f"t2{nm}{sb}_{h}")
            if swap_eng == "act":
                nc.scalar.copy(sw[0:64, :], psrc[64:128, :])
                nc.scalar.copy(sw[64:128, :], psrc[0:64, :])
            else:
                nc.vector.tensor_copy(sw[0:64, :], psrc[64:128, :])
                nc.vector.tensor_copy(sw[64:128, :], psrc[0:64, :])
            nc.vector.tensor_mul(t1[:], psrc[:], cos_sb[:])
            nc.vector.tensor_mul(t2[:], sw[:], sin_sb[:])
            ds = slice(h * S + sb * SBK, h * S + (sb + 1) * SBK)
            nc.vector.tensor_add(dst[:, ds], t1[:], t2[:])

        def emit_w_chunk(dst_sb, src_d, g):
            nc.gpsimd.dma_start(
                dst_sb[:, g * 2 * CW:(g + 1) * 2 * CW].rearrange(
                    "p (c m) -> p c m", m=CW),
                src_d[g * 256:(g + 1) * 256, :].rearrange(
                    "(c p) m -> p c m", p=128))

        pending = []  # deferred (lr, qs) epilogues, flushed one block later

        def flush_epilogue():
            for lr_p, qs_p, nm in pending:
                bcps = ps.tile([128, SBK], f32, tag="ps", name=f"bc{nm}")
                nc.tensor.matmul(bcps[:], lhsT=ones_row, rhs=lr_p[:],
                                 start=True, stop=True)
                nc.vector.tensor_mul(ot[:, qs_p], ot[:, qs_p], bcps[:])
            pending.clear()

        d_queue = []   # (q_i, eb, h) output-projection steps, one matmul each
        dstate = {}
        # iteration-0's C has no previous output projection to interleave;
        # iteration 1's Q-sweep (x prefetched, weights resident) fills it
        q_queue = []   # (h, ec) steps of the prequeued next-block Q sweep
        qstate = {}

        def emit_q_step(step):
            h, ec = step
            if ec == 0:
                qstate[h] = ps.tile([128, SBK], f32, tag="ps", name=f"qp1_{h}")
            g, el = ec // 2, ec % 2
            xv = qstate["xts"][g][:, el * SBK:(el + 1) * SBK]
            wsl = slice(ec * CW + h * HD, ec * CW + (h + 1) * HD)
            nc.tensor.matmul(qstate[h][:], lhsT=wq_sb[:, wsl], rhs=xv,
                             start=(ec == 0), stop=(ec == NEC - 1))
            if ec == NEC - 1:
                rope(qstate[h], qt, qstate["cos"], qstate["sin"],
                     1, h, "q", "dve")

        def emit_d_step(step):
            q_i, eb, h = step
            if eb == 0 and h == 0:
                dstate["z_sb"] = zp.tile([128, EMBED], f16, tag="zs",
                                         name=f"zs{q_i}")
            if h == 0:
                dstate["zps"] = ps.tile([128, SBK], f32, tag="ps",
                                        name=f"z{q_i}_{eb}")
            nc.tensor.matmul(
                dstate["zps"][:],
                lhsT=ot[:, h * S + q_i * 128:h * S + (q_i + 1) * 128],
                rhs=wo_sb[:, h * EMBED + eb * SBK:h * EMBED + (eb + 1) * SBK],
                start=(h == 0), stop=(h == HPC - 1))
            if h == HPC - 1:
                ebs = slice(eb * SBK, (eb + 1) * SBK)
                if eb % 2 == 0:
                    nc.vector.tensor_copy(dstate["z_sb"][:, ebs],
                                          dstate["zps"][:])
                else:
                    nc.scalar.copy(dstate["z_sb"][:, ebs], dstate["zps"][:])
                if q_i == NST - 1:
                    # last tile: store per chunk so the final drain is short
                    nc.sync.dma_start(z_d[q_i * 128:(q_i + 1) * 128, ebs],
                                      dstate["z_sb"][:, ebs])
                elif eb == 3:
                    nc.sync.dma_start(z_d[q_i * 128:(q_i + 1) * 128, :],
                                      dstate["z_sb"][:])

        for sb in range(NSB):
            ss = slice(sb * SBK, (sb + 1) * SBK)
            # ---- x tiles, loaded once and shared by Q/K/V sweeps ----
            if sb == 1:
                xts = qstate["xts"]  # prefetched during iteration 0
            else:
                xts = []
            for g in range(XG if sb != 1 else 0):
                xt_g = xa.tile([128, XW], f16, tag="x", name=f"x{sb}_{g}")
                if sb == 0 and g == 0:
                    # split the first loads so the first matmul starts early
                    for c in range(2):
                        nc.sync.dma_start(
                            xt_g[:, c * SBK:(c + 1) * SBK],
                            xt_d[c * 128:(c + 1) * 128, ss])
                        nc.gpsimd.dma_start(
                            wq_sb[:, c * CW:(c + 1) * CW],
                            wq_d[c * 128:(c + 1) * 128, :])
                else:
                    src_ap = xt_d[g * 256:(g + 1) * 256, ss]
                    nc.sync.dma_start(
                        xt_g[:].rearrange("p (c s) -> p c s", s=SBK),
                        src_ap.rearrange("(c p) s -> p c s", p=128))
                    if sb == 0:
                        emit_w_chunk(wq_sb, wq_d, g)
                xts.append(xt_g)
            if sb == 1:
                cos_sb, sin_sb = qstate["cos"], qstate["sin"]
            else:
                cos_sb = cs.tile([128, SBK], f16, tag="cos", name=f"cos{sb}")
                sin_sb = cs.tile([128, SBK], f16, tag="sin", name=f"sin{sb}")
                nc.gpsimd.dma_start(cos_sb[:], cos_d[:, ss])
                nc.gpsimd.dma_start(sin_sb[:], sin_d[:, ss])
            if sb == 0:
                nc.gpsimd.dma_start(msk_sb[:], msk_d[:])
                nc.gpsimd.dma_start(on32_sb[:], on32_d[:].bitcast(f32r))
            # -------- Q sweep (iteration 1's ran inside C(0)) --------
            if sb != 1:
                qp = [ps.tile([128, SBK], f32, tag="ps", name=f"qp{sb}_{_h}")
                      for _h in range(HPC)]
                for g in range(XG):
                    for el in range(2):
                        ec = 2 * g + el
                        st, sp = (ec == 0), (ec == NEC - 1)
                        xv = xts[g][:, el * SBK:(el + 1) * SBK]
                        for h in range(HPC):
                            wsl = slice(ec * CW + h * HD,
                                        ec * CW + (h + 1) * HD)
                            nc.tensor.matmul(qp[h][:], lhsT=wq_sb[:, wsl],
                                             rhs=xv, start=st, stop=sp)
                for h in range(HPC):
                    rope(qp[h], qt, cos_sb, sin_sb, sb, h, "q", "act")
            # -------- K sweep --------
            kp = [ps.tile([128, SBK], f32, tag="ps", name=f"kp{sb}_{_h}")
                  for _h in range(HPC)]
            for g in range(XG):
                if sb == 0:
                    emit_w_chunk(wk_sb, wk_d, g)
                for el in range(2):
                    ec = 2 * g + el
                    st, sp = (ec == 0), (ec == NEC - 1)
                    xv = xts[g][:, el * SBK:(el + 1) * SBK]
                    for h in range(HPC):
                        wsl = slice(ec * CW + h * HD, ec * CW + (h + 1) * HD)
                        nc.tensor.matmul(kp[h][:], lhsT=wk_sb[:, wsl],
                                         rhs=xv, start=st, stop=sp)
            for h in range(HPC):
                rope(kp[h], kt, cos_sb, sin_sb, sb, h, "k", "dve")
            # -------- V sweep (same x tiles) --------
            vp = [ps.tile([128, CW], f32, tag="ps", name=f"vp{sb}_{_s}")
                  for _s in range(4)]
            for g in range(XG):
                if sb == 0:
                    emit_w_chunk(wv_sb, wv_d, g)
                for el in range(2):
                    ec = 2 * g + el
                    st, sp = (ec == 0), (ec == NEC - 1)
                    for sub in range(4):
                        nc.tensor.matmul(
                            vp[sub][:],
                            lhsT=xts[g][:, el * SBK + sub * 128:
                                        el * SBK + (sub + 1) * 128],
                            rhs=wv_sb[:, ec * CW:(ec + 1) * CW],
                            start=st, stop=sp)
            for sub in range(4):
                stile = sb * 4 + sub
                dsl = vt[:, stile * CW:(stile + 1) * CW]
                if sub % 2 == 0:
                    nc.scalar.copy(dsl, vp[sub][:])
                else:
                    nc.vector.tensor_copy(dsl, vp[sub][:])
            if sb == 0:
                for h in range(HPC):
                    nc.gpsimd.dma_start(
                        wo_sb[:, h * EMBED:(h + 1) * EMBED],
                        wo_d[h * 128:(h + 1) * 128, :])
                # prefetch iteration 1's x tiles + rope tables and queue its
                # Q sweep as C(0)'s interleave filler
                ss1 = slice(SBK, 2 * SBK)
                xts1 = []
                for g in range(XG):
                    xt_g = xa.tile([128, XW], f16, tag="x", name=f"x1_{g}")
                    src_ap = xt_d[g * 256:(g + 1) * 256, ss1]
                    nc.sync.dma_start(
                        xt_g[:].rearrange("p (c s) -> p c s", s=SBK),
                        src_ap.rearrange("(c p) s -> p c s", p=128))
                    xts1.append(xt_g)
                qstate["xts"] = xts1
                c1 = cs.tile([128, SBK], f16, tag="cos", name="cos1")
                s1 = cs.tile([128, SBK], f16, tag="sin", name="sin1")
                nc.gpsimd.dma_start(c1[:], cos_d[:, ss1])
                nc.gpsimd.dma_start(s1[:], sin_d[:, ss1])
                qstate["cos"], qstate["sin"] = c1, s1
                q_queue = [(h, ec) for h in range(HPC) for ec in range(NEC)]

            # -------- attention blocks (h = 0..3, j = sb) --------
            # the previous iteration's output projection (d_queue) is
            # interleaved between attention tiles: per tile the PE's S+AV
            # (~430ns) trails the exp on ACT (~570ns), so without filler
            # the PE drains its psum-ring lead and stalls at ACT pace
            j = sb
            nkt = 4 * j + 4  # causal: k tiles 0..4j+3
            # the pending h=3 epilogue of iteration sb-1 must flush before
            # any interleaved D matmul touches that head's normalized O^T
            # (its broadcast would otherwise sit behind the reader in the
            # in-order PE queue)
            flush_epilogue()
            fill_q = q_queue if q_queue else d_queue
            d_ratio = len(fill_q) / float(HPC * nkt) if fill_q else 0.0
            d_credit = 0.0
            fill_fn = emit_q_step if q_queue else emit_d_step
            for h in range(HPC):
                avp = ps.tile([128, SBK], f32, tag="ps", name=f"av{h}_{j}")
                u_sb = upl.tile([128, SBK], f16, tag="u", name=f"u{h}_{j}")
                qs = slice(h * S + j * SBK, h * S + (j + 1) * SBK)
                pt_last = None
                pend_av = []  # software-pipeline AV behind S

                def do_av(pi, pq0, ppt):
                    nonlocal pt_last
                    nc.tensor.matmul(
                        avp[:, pq0:SBK],
                        lhsT=vt[:, pi * CW + h * HD:pi * CW + (h + 1) * HD],
                        rhs=ppt[:, pq0:SBK],
                        start=(pi == 0), stop=(pi == nkt - 1))
                    # fold P into the running denominator sum on DVE; the
                    # last (128-wide) diagonal tile goes straight into a
                    # second l-matmul instead, off the serial DVE chain
                    if pi == 0:
                        nc.vector.tensor_copy(u_sb[:], ppt[:])
                    elif pi < nkt - 1:
                        nc.vector.tensor_add(u_sb[:, pq0:SBK],
                                             u_sb[:, pq0:SBK],
                                             ppt[:, pq0:SBK])
                    else:
                        pt_last = ppt

                for i in range(nkt):
                    o_idx = i - 4 * j
                    # crossing tiles: only q >= 128*o_idx is unmasked
                    q0 = 128 * o_idx if o_idx > 0 else 0
                    sp_t = ps.tile([128, SBK], f32, tag="ps",
                                   name=f"s{h}_{j}_{i}")
                    ks = slice(h * S + i * 128, h * S + (i + 1) * 128)
                    nc.tensor.matmul(sp_t[:, q0:SBK], lhsT=kt[:, ks],
                                     rhs=qt[:, qs.start + q0:qs.stop],
                                     start=True, stop=True)
                    pt_sb = ptp.tile([128, SBK], f16, tag="p",
                                     name=f"p{h}_{j}_{i}")
                    nc.scalar.activation(pt_sb[:, q0:SBK], sp_t[:, q0:SBK],
                                         EXP, scale=SCALE)
                    if o_idx >= 0:  # mask the diagonal 128-band
                        nc.vector.tensor_mul(
                            pt_sb[:, q0:q0 + 128], pt_sb[:, q0:q0 + 128], tri)
                    if len(pend_av) == 2:  # AV runs two tiles behind S
                        do_av(*pend_av.pop(0))
                    pend_av.append((i, q0, pt_sb))
                    d_credit += d_ratio
                    while fill_q and d_credit >= 1.0:
                        fill_fn(fill_q.pop(0))
                        d_credit -= 1.0
                for pa in pend_av:
                    do_av(*pa)
                # denominators: fold partitions into a psum row
                lp = ps.tile([1, SBK], f32, tag="ps", name=f"l{h}_{j}")
                nc.tensor.matmul(lp[:], lhsT=ones_col, rhs=u_sb[:],
                                 start=True, stop=False)
                nc.tensor.matmul(lp[:, 384:SBK], lhsT=ones_col,
                                 rhs=pt_last[:, 384:SBK],
                                 start=False, stop=True)
                lr = lrp.tile([1, SBK], f32, tag="lr", name=f"lr{h}_{j}")
                nc.vector.reciprocal_approx_fast(lr[:], lp[:])
                lrh = lrp.tile([1, SBK], f16, tag="lrh", name=f"lrh{h}_{j}")
                nc.vector.tensor_copy(lrh[:], lr[:])
                nc.vector.tensor_copy(ot[:, qs], avp[:])
                # broadcast+normalize of the PREVIOUS block runs here so its
                # PE matmul never waits on this block's reciprocal
                flush_epilogue()
                pending.append((lrh, qs, f"{h}_{j}"))

            # -------- output projection --------
            # drain leftover filler, then queue D(sb) for interleaving into
            # the next iteration's attention tiles; the last iteration's D
            # runs here directly (no C to hide it in)
            while q_queue:
                emit_q_step(q_queue.pop(0))
            while d_queue:
                emit_d_step(d_queue.pop(0))
            if sb < NSB - 1:
                d_queue = [(q_i, eb, h)
                           for q_i in range(4 * sb, 4 * sb + 4)
                           for eb in range(4)
                           for h in range(HPC)]
            else:
                for q_i in range(4 * sb, 4 * sb + 4):
                    for eb in range(4):
                        for h in range(HPC):
                            if h == HPC - 1 and pending:
                                flush_epilogue()
                            emit_d_step((q_i, eb, h))

        vmp.release()
        zp.release()
        upl.release()
        ptp.release()
        lrp.release()
        rp.release()
        cs.release()
        wop.release()
        wp.release()
        wvp.release()
        xa.release()
        pp.release()
        ps.release()

    nc.compile()
    return nc


def _host_tables():
    inv_freq = 1.0 / (ROPE_BASE ** (np.arange(0, HD, 2, dtype=np.float64) / HD))
    ang = np.arange(S, dtype=np.float64)[:, None] * inv_freq[None, :]  # [S, 64]
    cos = np.cos(ang)
    sin = np.sin(ang)
    cost = np.ascontiguousarray(
        np.concatenate([cos, cos], axis=1).T.astype(np.float16))  # [128, S]
    sints = np.ascontiguousarray(
        np.concatenate([-sin, sin], axis=1).T.astype(np.float16))
    kk = np.arange(128)[:, None]
    rr = np.arange(128)[None, :]
    masks = np.ones((128, 256), dtype=np.float16)
    masks[:, 0:128] = (kk <= rr).astype(np.float16)
    return cost, sints, masks


def _in_maps(x, Wq, Wk, Wv, Wo):
    cost, sints, masks = _host_tables()
    ones32 = np.ones((1, 128), dtype=np.float32)
    maps = []
    for c in range(N_CORES):
        b = c // 4
        h0 = (c % 4) * CW  # column offset of this core's 4 heads
        maps.append({
            "xt": np.ascontiguousarray(x[b].T.astype(np.float16)),
            "wq": np.ascontiguousarray(Wq[:, h0:h0 + CW].astype(np.float16)),
            "wk": np.ascontiguousarray(Wk[:, h0:h0 + CW].astype(np.float16)),
            "wv": np.ascontiguousarray(Wv[:, h0:h0 + CW].astype(np.float16)),
            "wo": np.ascontiguousarray(Wo[h0:h0 + CW, :].astype(np.float16)),
            "cost": cost,
            "sints": sints,
            "masks": masks,
            "ones32": ones32,
        })
    return maps


def kernel(x, Wq, Wk, Wv, Wo):
    from concourse.bass_utils import run_bass_kernel_spmd

    x = np.asarray(x, dtype=np.float32)
    Wq = np.asarray(Wq, dtype=np.float32)
    Wk = np.asarray(Wk, dtype=np.float32)
    Wv = np.asarray(Wv, dtype=np.float32)
    Wo = np.asarray(Wo, dtype=np.float32)

    if "nc" not in _CACHE:
        _CACHE["nc"] = _build_program()
    nc = _CACHE["nc"]

    res = run_bass_kernel_spmd(nc, _in_maps(x, Wq, Wk, Wv, Wo),
                               core_ids=list(range(N_CORES)))
    zs = [np.asarray(res.results[c]["z"], dtype=np.float32)
          for c in range(N_CORES)]
    out = np.empty((B, S, EMBED), dtype=np.float32)
    out[0] = zs[0] + zs[1] + zs[2] + zs[3]
    out[1] = zs[4] + zs[5] + zs[6] + zs[7]
    return out
